# revision 19
# baseline (speedup 1.0000x reference)
"""SwinV2 forward on 8 Trainium2 NeuronCores, pure data parallel (1 image/core).

Self-contained: hardcodes all shapes. Device kernel written in Bass/Tile.
 - Activation stream lives in DRAM between phases, padded (H+4, W+4, C) with a
   wrap halo so shifted-window gathers/scatters are single rectangular DMAs.
 - Tokens-major compute tiles (<=128 tokens on partitions, channels free),
   processed in chunk-groups of <=8 x 128 tokens.
 - Dense matmuls keep fp32 data typed float32r (full PE rate at N>=256).
 - Attention internals (qn/kn/v/P) are bf16.
 - Softmax: exp(S-26) with CPB bias and shift mask folded in multiplicatively.
"""
import numpy as np
from contextlib import ExitStack

import concourse.bass as bass
import concourse.bacc as bacc
import concourse.mybir as mybir
import concourse.tile as tile
from concourse.masks import make_identity
from concourse import bass_utils

F32 = mybir.dt.float32
F32R = mybir.dt.float32r
BF16 = mybir.dt.bfloat16
U16 = mybir.dt.uint16
AF = mybir.ActivationFunctionType
OP = mybir.AluOpType
AX = mybir.AxisListType

DEPTHS = (2, 2, 6, 2)
HEADS = (3, 6, 12, 24)
DIMS = (96, 192, 384, 768)
HS = (64, 32, 16, 8)
WS = 8
PAD = 4
NCORES = 8
LN100 = float(np.log(100.0))
ESH = 26.0


def cdiv(a, b):
    return (a + b - 1) // b


# ----------------------------------------------------------------- host consts
def _rpi(ws):
    c = np.stack(np.meshgrid(np.arange(ws), np.arange(ws), indexing='ij'))
    cf = c.reshape(2, -1)
    rel = (cf[:, :, None] - cf[:, None, :]).transpose(1, 2, 0).astype(np.int64)
    rel[:, :, 0] += ws - 1
    rel[:, :, 1] += ws - 1
    rel[:, :, 0] *= 2 * ws - 1
    return rel.sum(-1)


def _coords_table(ws):
    h = np.arange(-(ws - 1), ws, dtype=np.float32)
    t = np.stack(np.meshgrid(h, h, indexing='ij'), axis=-1)
    t = t / (ws - 1) * 8.0
    t = np.sign(t) * np.log2(np.abs(t) + 1.0) / np.log2(8.0)
    return t.reshape(-1, 2).astype(np.float32)


def _attn_mask(H, W, ws, shift):
    img = np.zeros((H, W))
    cnt = 0
    for hs in (slice(0, -ws), slice(-ws, -shift), slice(-shift, None)):
        for wsl in (slice(0, -ws), slice(-ws, -shift), slice(-shift, None)):
            img[hs, wsl] = cnt
            cnt += 1
    mw = img.reshape(H // ws, ws, W // ws, ws).transpose(0, 2, 1, 3).reshape(-1, ws * ws)
    d = mw[:, None, :] - mw[:, :, None]
    return np.where(d != 0, -100.0, 0.0).astype(np.float32)


def host_constants():
    rpi = _rpi(WS).astype(np.int64).reshape(-1)
    gmat = np.zeros((225, 4096), np.float32)
    gmat[rpi, np.arange(4096)] = 1.0
    consts = {"gmat": gmat, "tblT": _coords_table(WS).T.copy().astype(np.float32)}
    for s in range(3):
        H = HS[s]
        nW = (H // WS) ** 2
        em = np.exp(_attn_mask(H, H, WS, WS // 2)).astype(np.float32)
        m01 = np.zeros((nW // 2, 128, 128), np.float32)
        for wp in range(nW // 2):
            m01[wp, 0:64, 0:64] = em[2 * wp]
            m01[wp, 64:128, 64:128] = em[2 * wp + 1]
        consts[f"m01_{s}"] = m01
    m01u = np.zeros((128, 128), np.float32)
    m01u[0:64, 0:64] = 1.0
    m01u[64:128, 64:128] = 1.0
    consts["m01u"] = m01u
    return consts


def flatten_params(params):
    A = lambda v: np.ascontiguousarray(np.asarray(v), dtype=np.float32)
    f = {}
    f["pewT"] = A(params["patch_w"]).reshape(96, 48).T.copy()
    f["peb"] = A(params["patch_b"])
    f["pelng"] = A(params["patch_ln_g"])
    f["pelnb"] = A(params["patch_ln_b"])
    for s in range(4):
        st = params["stages"][s]
        C = DIMS[s]
        for b in range(DEPTHS[s]):
            bp = st["blocks"][b]
            p = f"s{s}b{b}_"
            wq, wk, wv = A(bp["wq"]), A(bp["wk"]), A(bp["wv"])
            f[p + "qkvT"] = np.concatenate([wq.T, wk.T, wv.T], axis=1).copy()
            f[p + "bqkv"] = np.concatenate([A(bp["bq"]), np.zeros(C, np.float32),
                                            A(bp["bv"])])
            f[p + "ls"] = A(bp["logit_scale"]).reshape(-1)
            f[p + "cpb1T"] = A(bp["cpb_w1"]).T.copy()
            f[p + "cpb1br"] = A(bp["cpb_b1"]).reshape(4, 128).T.copy()
            f[p + "cpb2T"] = A(bp["cpb_w2"]).T.copy()
            f[p + "projT"] = A(bp["proj_w"]).T.copy()
            f[p + "projb"] = A(bp["proj_b"])
            f[p + "ln1g"], f[p + "ln1b"] = A(bp["ln1_g"]), A(bp["ln1_b"])
            f[p + "ln2g"], f[p + "ln2b"] = A(bp["ln2_g"]), A(bp["ln2_b"])
            f[p + "fc1T"] = A(bp["fc1_w"]).T.copy()
            f[p + "fc1b"] = A(bp["fc1_b"])
            f[p + "fc2T"] = A(bp["fc2_w"]).T.copy()
            f[p + "fc2b"] = A(bp["fc2_b"])
        if s < 3:
            f[f"mrg{s}_redT"] = A(st["red_w"]).T.copy()
            f[f"mrg{s}_g"] = A(st["ds_ln_g"])
            f[f"mrg{s}_b"] = A(st["ds_ln_b"])
    f["fing"], f["finb"] = A(params["final_ln_g"]), A(params["final_ln_b"])
    f["headT"] = A(params["head_w"]).T.copy()
    f["headbr"] = A(params["head_b"]).reshape(4, 128).T.copy()
    return f



def tap(t):
    """Full-view AP of a pool tile."""
    return t[tuple(slice(None) for _ in t.shape)]


# ------------------------------------------------------------------ device IR
class Swin:
    def __init__(self):
        self.nc = bacc.Bacc("TRN2", target_bir_lowering=False)
        self.D = {}
        self.sqp = None
        self.hseq = 0

    def dram_in(self, name, shape, dt=F32):
        self.D[name] = self.nc.dram_tensor(name, list(shape), dt, kind="ExternalInput")

    def dram_out(self, name, shape, dt=F32):
        self.D[name] = self.nc.dram_tensor(name, list(shape), dt, kind="ExternalOutput")

    def dram_tmp(self, name, shape, dt=F32):
        self.D[name] = self.nc.dram_tensor(name, list(shape), dt)

    def brep(self, dram, n, pool, tag, dt=F32):
        t = pool.tile([128, n], dt, tag=tag)
        self.nc.sync.dma_start(out=t, in_=bass.AP(tensor=dram, offset=0,
                                                  ap=[[0, 128], [1, n]]))
        return t

    def const_tile(self, pool, val, tag):
        t = pool.tile([128, 1], F32, tag=tag)
        self.nc.vector.memset(t, val)
        return t

    def ln_cg(self, big, PT, n, C, gR, bR, sm, sq, eps_t):
        """In-place LN over last dim C of big view (PT, n, C)."""
        nc = self.nc
        mean = sm.tile([128, 8], F32, tag="lnm")
        msq = sm.tile([128, 8], F32, tag="lnq")
        nc.vector.tensor_reduce(out=mean[0:PT, 0:n], in_=big, axis=AX.X, op=OP.add)
        sqv = sq.tile([128, 3072], F32, tag="sq")
        sq_v = bass.AP(tensor=sqv.tensor, offset=tap(sqv).offset,
                       ap=[[tap(sqv).ap[0][0], PT], [C, n], [1, C]])
        nc.scalar.activation(sq_v, big, AF.Square)
        nc.vector.tensor_reduce(out=msq[0:PT, 0:n], in_=sq_v, axis=AX.X, op=OP.add)
        nc.vector.tensor_scalar_mul(out=mean[0:PT, 0:n], in0=mean[0:PT, 0:n],
                                    scalar1=1.0 / C)
        nc.vector.tensor_scalar_mul(out=msq[0:PT, 0:n], in0=msq[0:PT, 0:n],
                                    scalar1=1.0 / C)
        var = sm.tile([128, 8], F32, tag="lnv")
        nc.vector.tensor_tensor(var[0:PT, 0:n], mean[0:PT, 0:n], mean[0:PT, 0:n],
                                OP.mult)
        nc.vector.tensor_tensor(var[0:PT, 0:n], msq[0:PT, 0:n], var[0:PT, 0:n],
                                OP.subtract)
        rstd = sm.tile([128, 8], F32, tag="lnr")
        nc.scalar.activation(rstd[0:PT, 0:n], var[0:PT, 0:n], AF.Sqrt,
                             bias=eps_t[0:PT, :])
        nc.vector.reciprocal(rstd[0:PT, 0:n], rstd[0:PT, 0:n])
        nc.vector.tensor_tensor(big, big,
                                mean[0:PT, 0:n, None].to_broadcast((PT, n, C)),
                                OP.subtract)
        nc.vector.tensor_tensor(big, big,
                                rstd[0:PT, 0:n, None].to_broadcast((PT, n, C)),
                                OP.mult)
        nc.vector.tensor_tensor(big, big,
                                gR[0:PT, None, :].to_broadcast((PT, n, C)), OP.mult)
        nc.vector.tensor_tensor(big, big,
                                bR[0:PT, None, :].to_broadcast((PT, n, C)), OP.add)

    def _dd_copy(self, st, dst_off, src_off, ap_dims, pool, nelem):
        """DRAM->DRAM copy bounced through SBUF (tracked deps)."""
        nc = self.nc
        t = pool.tile([128, 3072], F32, tag="sq", name=f"hb{self.hseq}")
        self.hseq += 1
        if ap_dims is None:
            k = nelem // 128
            tv = t[0:128, 0:k]
            nc.sync.dma_start(out=tv, in_=bass.AP(tensor=st, offset=src_off,
                                                  ap=[[k, 128], [1, k]]))
            nc.sync.dma_start(out=bass.AP(tensor=st, offset=dst_off,
                                          ap=[[k, 128], [1, k]]), in_=tv)
        else:
            npart, nfree = ap_dims[0][1], ap_dims[1][1]
            tv = t[0:npart, 0:nfree]
            nc.sync.dma_start(out=tv, in_=bass.AP(tensor=st, offset=src_off,
                                                  ap=ap_dims))
            nc.sync.dma_start(out=bass.AP(tensor=st, offset=dst_off, ap=ap_dims),
                              in_=tv)

    def halo_fix(self, st, H, C):
        Wp = H + PAD
        if PAD * C <= 3072:
            self._dd_copy(st, H * C, 0, [[Wp * C, H], [1, PAD * C]], self.sqp,
                          H * PAD * C)
        else:
            for j in range(PAD):
                self._dd_copy(st, (H + j) * C, j * C, [[Wp * C, H], [1, C]],
                              self.sqp, H * C)
        self._dd_copy(st, H * Wp * C, 0, None, self.sqp, PAD * Wp * C)

    def halo_unfix(self, st, H, C):
        Wp = H + PAD
        self._dd_copy(st, 0, H * Wp * C, None, self.sqp, PAD * Wp * C)
        if PAD * C <= 3072:
            self._dd_copy(st, 0, H * C, [[Wp * C, H], [1, PAD * C]], self.sqp,
                          H * PAD * C)
        else:
            for j in range(PAD):
                self._dd_copy(st, j * C, (H + j) * C, [[Wp * C, H], [1, C]],
                              self.sqp, H * C)

    def dense_mm(self, lhsT_tiles, w_dram, Cin, Nout, ps, wts, evict):
        """out[ci] = lhsT_tiles[ci].T @ W for each chunk; evict(ci, nt, psum_ap).
        lhsT_tiles: per chunk list of per-kchunk (tile_ap, PT). Splits chunks into
        psum-safe sub-batches internally."""
        nc = self.nc
        Kc = cdiv(Cin, 128)
        NT = cdiv(Nout, 512)
        nch = len(lhsT_tiles)
        sub = max(1, min(nch, 6 // NT))
        for s0 in range(0, nch, sub):
            chs = list(range(s0, min(s0 + sub, nch)))
            psl = {}
            for ci in chs:
                for nt in range(NT):
                    psl[(ci, nt)] = ps.tile([128, 512], F32, tag="ps", name=f"ps{ci}_{nt}")
            for nt in range(NT):
                n0, n1 = nt * 512, min((nt + 1) * 512, Nout)
                for kch in range(Kc):
                    k0, k1 = kch * 128, min((kch + 1) * 128, Cin)
                    wt = wts.tile([128, 512], F32R, tag="wt")
                    nc.sync.dma_start(
                        out=wt[0:k1 - k0, 0:n1 - n0],
                        in_=bass.AP(tensor=w_dram, offset=k0 * Nout + n0,
                                    ap=[[Nout, k1 - k0], [1, n1 - n0]]))
                    for ci in chs:
                        t, PT = lhsT_tiles[ci][kch]
                        nc.tensor.matmul(
                            psl[(ci, nt)][0:PT, 0:n1 - n0],
                            t, wt[0:k1 - k0, 0:n1 - n0],
                            start=(kch == 0), stop=(kch == Kc - 1))
            for ci in chs:
                for nt in range(NT):
                    PT = lhsT_tiles[ci][0][1]
                    n0, n1 = nt * 512, min((nt + 1) * 512, Nout)
                    evict(ci, nt, psl[(ci, nt)][0:PT, 0:n1 - n0])

    def transpose_to(self, src_ap, PT, cols, ident, ps, outpool, dt, tag):
        nc = self.nc
        o = outpool.tile([128, 128], dt, tag=tag)
        if dt == BF16:
            pt = ps.tile([128, 256], BF16, tag="ps")
        else:
            pt = ps.tile([128, 512], F32, tag="ps")
        nc.tensor.transpose(pt[0:cols, 0:PT], src_ap, ident[0:PT, 0:PT])
        nc.scalar.copy(o[0:cols, 0:PT], pt[0:cols, 0:PT])
        return o


def build_nc():
    K = Swin()
    nc = K.nc
    D = K.D

    K.dram_in("xp", (48, 4096), F32R)
    K.dram_in("gmat", (225, 4096), BF16)
    K.dram_in("tblT", (2, 225))
    for s in range(3):
        nW = (HS[s] // WS) ** 2
        K.dram_in(f"m01_{s}", (nW // 2, 128, 128), BF16)
    K.dram_in("m01u", (128, 128), BF16)
    K.dram_in("pewT", (48, 96), F32R)
    K.dram_in("peb", (96,))
    K.dram_in("pelng", (96,))
    K.dram_in("pelnb", (96,))
    for s in range(4):
        C, h = DIMS[s], HEADS[s]
        for b in range(DEPTHS[s]):
            p = f"s{s}b{b}_"
            K.dram_in(p + "qkvT", (C, 3 * C), F32R)
            K.dram_in(p + "bqkv", (3 * C,))
            K.dram_in(p + "ls", (h,))
            K.dram_in(p + "cpb1T", (2, 512))
            K.dram_in(p + "cpb1br", (128, 4))
            K.dram_in(p + "cpb2T", (512, h))
            K.dram_in(p + "projT", (C, C), F32R)
            K.dram_in(p + "projb", (C,))
            K.dram_in(p + "ln1g", (C,)); K.dram_in(p + "ln1b", (C,))
            K.dram_in(p + "ln2g", (C,)); K.dram_in(p + "ln2b", (C,))
            K.dram_in(p + "fc1T", (C, 4 * C), F32R)
            K.dram_in(p + "fc1b", (4 * C,))
            K.dram_in(p + "fc2T", (4 * C, C), F32R)
            K.dram_in(p + "fc2b", (C,))
        if s < 3:
            K.dram_in(f"mrg{s}_redT", (4 * C, 2 * C), F32R)
            K.dram_in(f"mrg{s}_g", (2 * C,))
            K.dram_in(f"mrg{s}_b", (2 * C,))
    K.dram_in("fing", (768,)); K.dram_in("finb", (768,))
    K.dram_in("headT", (768, 512)); K.dram_in("headbr", (128, 4))

    for s in range(4):
        Wp = HS[s] + PAD
        K.dram_out(f"st{s}", (Wp * Wp * DIMS[s],))
        K.dram_out(f"ao{s}", (Wp * Wp * DIMS[s],))
    K.dram_tmp("ebS", (24, 4096), BF16)
    K.dram_out("o_head", (128, 4))
    K.dram_out("o_p0", (128, 32))
    K.dram_out("o_p1", (128, 8))
    K.dram_out("o_p2", (128, 2))
    K.dram_out("o_p3", (64, 1))

    with tile.TileContext(nc) as tc, ExitStack() as ctx:
        cst = ctx.enter_context(tc.tile_pool(name="cst", bufs=1))
        wts = ctx.enter_context(tc.tile_pool(name="wts", bufs=3))
        brp = ctx.enter_context(tc.tile_pool(name="brp", bufs=1))
        bcp = ctx.enter_context(tc.tile_pool(name="bcp", bufs=4))
        sm = ctx.enter_context(tc.tile_pool(name="sm", bufs=2))
        smI = ctx.enter_context(tc.tile_pool(name="smI", bufs=1))
        sq = ctx.enter_context(tc.tile_pool(name="sq", bufs=1))
        K.sqp = sq
        ps = ctx.enter_context(tc.tile_pool(name="ps", bufs=8, space="PSUM"))
        xtp = ctx.enter_context(tc.tile_pool(name="xtp", bufs=8))
        xgp = ctx.enter_context(tc.tile_pool(name="xgp", bufs=3))
        ptp = ctx.enter_context(tc.tile_pool(name="ptp", bufs=4))
        ebp = ctx.enter_context(tc.tile_pool(name="ebp", bufs=1))
        pop = ctx.enter_context(tc.tile_pool(name="pop", bufs=2))

        identf = cst.tile([128, 128], F32, tag="idf")
        make_identity(nc, identf[:, :])
        identb = cst.tile([128, 128], BF16, tag="idb")
        make_identity(nc, identb[:, :])
        gmt0 = cst.tile([128, 4096], BF16, tag="gm0")
        nc.sync.dma_start(out=gmt0[0:113, :], in_=D["gmat"][0:113, :])
        gmt1 = cst.tile([128, 4096], BF16, tag="gm1")
        nc.sync.dma_start(out=gmt1[0:112, :], in_=D["gmat"][113:225, :])
        tblt = cst.tile([2, 225], F32, tag="tbl")
        nc.sync.dma_start(out=tblt, in_=D["tblT"][:, :])
        m01t = {}
        for s in range(3):
            nWp = (HS[s] // WS) ** 2 // 2
            m01t[s] = cst.tile([128, nWp, 128], BF16, tag=f"m01_{s}", name=f"m01t{s}")
            nc.sync.dma_start(out=m01t[s],
                              in_=D[f"m01_{s}"][:, :, :].rearrange("w p k -> p w k"))
        m01ut = cst.tile([128, 128], BF16, tag="m01u")
        nc.sync.dma_start(out=m01ut, in_=D["m01u"][:, :])
        neg26 = K.const_tile(cst, -ESH, "n26")
        one16 = K.const_tile(cst, 16.0, "o16")
        eps_t = K.const_tile(cst, 1e-5, "eps")
        onesc = K.const_tile(cst, 1.0, "one")

        # ============ patch embed ============
        with tc.tile_pool(name="pe", bufs=2) as PEP:
            C = 96
            H, Wp = 64, 68
            pebR = K.brep(D["peb"], C, brp, "bC")
            gR = K.brep(D["pelng"], C, bcp, "bC2")
            bR = K.brep(D["pelnb"], C, bcp, "bC2")
            p0 = smI.tile([128, 32], F32, tag="pool")
            for g in range(4):
                cg = list(range(g * 8, g * 8 + 8))
                big = PEP.tile([128, 8, C], F32, tag="peb")
                lts = []
                for ci in cg:
                    xt = xtp.tile([128, 128], F32R, tag="xt")
                    nc.sync.dma_start(out=xt[0:48, 0:128],
                                      in_=D["xp"][:, ci * 128:(ci + 1) * 128])
                    lts.append([(xt[0:48, 0:128], 128)])

                def ev(lci, nt, pap, big=big, cg=cg):
                    nc.vector.tensor_tensor(big[:, lci, 0:96], pap[:, 0:96],
                                            pebR[:, 0:96], OP.add)
                K.dense_mm(lts, D["pewT"], 48, 96, ps, wts, ev)
                K.ln_cg(big[:, :, :], 128, 8, C, gR, bR, sm, sq, eps_t)
                nc.vector.tensor_reduce(out=p0[:, g * 8:(g + 1) * 8],
                                        in_=big[:, :, :], axis=AX.X, op=OP.add)
                for lci, ci in enumerate(cg):
                    nc.sync.dma_start(
                        out=bass.AP(tensor=D["st0"], offset=2 * ci * Wp * C,
                                    ap=[[Wp * C, 2], [1, 64 * C]]),
                        in_=big[:, lci, :])
            nc.vector.tensor_scalar_mul(out=p0[:, :], in0=p0[:, :], scalar1=1.0 / C)
            nc.sync.dma_start(out=D["o_p0"][:, :], in_=p0[:, :])
            K.halo_fix(D["st0"], 64, C)

        # ============ stages ============
        for s in range(4):
            C, h, H = DIMS[s], HEADS[s], HS[s]
            T = H * H
            PT = min(128, T)
            Tc = max(1, T // 128)
            WPC = PT // 64
            Cc = cdiv(C, 128)
            C4c = cdiv(4 * C, 128)
            Wp = H + PAD
            nWrow = H // WS
            rpc = PT // H  # raster rows per chunk
            CG = {0: 8, 1: 4, 2: 2, 3: 1}[s]
            st, ao = D[f"st{s}"], D[f"ao{s}"]
            with tc.tile_pool(name=f"stg{s}", bufs=1) as SP, \
                 tc.tile_pool(name=f"stg{s}d", bufs=2) as SP2:
                for b in range(DEPTHS[s]):
                    p = f"s{s}b{b}_"
                    shift = (WS // 2) if (b % 2 == 1 and H > WS) else 0

                    # ---------- CPB -> exp(bias) ----------
                    w1 = sm.tile([2, 512], F32, tag="cpw1")
                    nc.sync.dma_start(out=w1, in_=D[p + "cpb1T"][:, :])
                    b1r = sm.tile([128, 4], F32, tag="cpb1")
                    nc.sync.dma_start(out=b1r, in_=D[p + "cpb1br"][:, :])
                    w2 = sm.tile([128, 4, 32], F32, tag="cpw2")
                    nc.sync.dma_start(
                        out=w2[:, :, 0:h],
                        in_=bass.AP(tensor=D[p + "cpb2T"], offset=0,
                                    ap=[[h, 128], [128 * h, 4], [1, h]]))
                    ch1 = sm.tile([128, 4, 225], F32, tag="cph1", bufs=1)
                    for m in range(4):
                        pt = ps.tile([128, 512], F32, tag="ps")
                        nc.tensor.matmul(pt[0:128, 0:225],
                                         w1[:, m * 128:(m + 1) * 128], tblt[:, :])
                        nc.scalar.activation(ch1[:, m, :], pt[0:128, 0:225], AF.Relu,
                                             bias=b1r[:, m:m + 1])
                    sgT = sm.tile([128, 2, 32], BF16, tag="cpsg")
                    for mc, (j0, j1) in enumerate(((0, 113), (113, 225))):
                        jn = j1 - j0
                        pt = ps.tile([128, 512], F32, tag="ps")
                        for m in range(4):
                            nc.tensor.matmul(pt[0:jn, 0:h], ch1[:, m, j0:j1],
                                             w2[:, m, 0:h],
                                             start=(m == 0), stop=(m == 3))
                        tmp = sm.tile([128, 32], F32, tag="cptmp")
                        nc.scalar.activation(tmp[0:jn, 0:h], pt[0:jn, 0:h], AF.Sigmoid)
                        nc.scalar.activation(sgT[0:jn, mc, 0:h], tmp[0:jn, 0:h],
                                             AF.Exp, scale=one16[0:jn, :])
                    ebH = smI.tile([128, 4096], BF16, tag="ebal")
                    for nt in range(8):
                        pq = ps.tile([128, 512], F32, tag="ps")
                        nc.tensor.matmul(pq[0:h, 0:512],
                                         sgT[0:113, 0, 0:h],
                                         gmt0[0:113, nt * 512:(nt + 1) * 512],
                                         start=True, stop=False)
                        nc.tensor.matmul(pq[0:h, 0:512],
                                         sgT[0:112, 1, 0:h],
                                         gmt1[0:112, nt * 512:(nt + 1) * 512],
                                         start=False, stop=True)
                        nc.scalar.copy(ebH[0:h, nt * 512:(nt + 1) * 512],
                                       pq[0:h, 0:512])
                    nc.sync.dma_start(out=D["ebS"][0:h, :], in_=ebH[0:h, :])
                    if s < 3:
                        ebd = []
                        for hh in range(h):
                            e = ebp.tile([128, 128], BF16, tag="ebd", bufs=13)
                            for half in range(2):
                                nc.sync.dma_start(
                                    out=e[64 * half:64 * half + 64, :],
                                    in_=bass.AP(tensor=D["ebS"], offset=hh * 4096,
                                                ap=[[64, 64], [0, 2], [1, 64]]))
                            ebd.append(e)
                    else:
                        eba = ebp.tile([64, 24, 64], BF16, tag="eba", bufs=1)
                        nc.sync.dma_start(
                            out=eba,
                            in_=bass.AP(tensor=D["ebS"], offset=0,
                                        ap=[[64, 64], [4096, 24], [1, 64]]))

                    bqR = K.brep(D[p + "bqkv"], 3 * C, brp, "b3C")
                    lsR = K.brep(D[p + "ls"], h, bcp, "bh")
                    scl = sm.tile([128, 32], F32, tag="scl")
                    nc.vector.tensor_scalar(out=scl[0:PT, 0:h], in0=lsR[0:PT, :],
                                            scalar1=LN100, scalar2=None, op0=OP.min)
                    nc.scalar.activation(scl[0:PT, 0:h], scl[0:PT, 0:h], AF.Exp)
                    pjbR = K.brep(D[p + "projb"], C, brp, "bC")

                    # ---------- attention by chunk-group ----------
                    for g0 in range(0, Tc, CG):
                        cg = list(range(g0, min(g0 + CG, Tc)))
                        ng = len(cg)
                        qkv = SP.tile([PT, CG, 3 * C], BF16, tag="qkvb")
                        # qkv matmuls
                        lts = []
                        for lci, ci in enumerate(cg):
                            xg = xgp.tile([128, 768], F32, tag="xg")
                            for w in range(WPC):
                                wr = (WPC * ci + w) // nWrow
                                wc = (WPC * ci + w) % nWrow
                                nc.sync.dma_start(
                                    out=xg[w * 64:(w + 1) * 64, 0:C],
                                    in_=bass.AP(
                                        tensor=st,
                                        offset=((wr * WS + shift) * Wp
                                                + wc * WS + shift) * C,
                                        ap=[[Wp * C, WS], [1, WS * C]]))
                            kts = []
                            for kch in range(Cc):
                                k0, k1 = kch * 128, min((kch + 1) * 128, C)
                                xt = K.transpose_to(xg[0:PT, k0:k1], PT, k1 - k0,
                                                    identf, ps, xtp, F32R, "xt")
                                kts.append((xt[0:k1 - k0, 0:PT], PT))
                            lts.append(kts)

                        def ev_qkv(lci, nt, pap, qkv=qkv):
                            n0 = nt * 512
                            n1 = min(n0 + 512, 3 * C)
                            nc.vector.tensor_tensor(qkv[0:PT, lci, n0:n1], pap,
                                                    bqR[0:PT, n0:n1], OP.add)
                        K.dense_mm(lts, D[p + "qkvT"], C, 3 * C, ps, wts, ev_qkv)

                        # norms + scale
                        nrm = sm.tile([128, 8, 48], F32, tag="nrm")
                        sqv = sq.tile([128, 8 * 768], F32, tag="sq")
                        sq_v = bass.AP(tensor=sqv.tensor, offset=tap(sqv).offset,
                                       ap=[[tap(sqv).ap[0][0], PT], [2 * C, ng],
                                           [1, 2 * C]])
                        qk_v = bass.AP(tensor=qkv.tensor, offset=tap(qkv).offset,
                                       ap=[[tap(qkv).ap[0][0], PT], [3 * C, ng],
                                           [1, 2 * C]])
                        nc.scalar.activation(sq_v, qk_v, AF.Square)
                        sq_h = bass.AP(tensor=sqv.tensor, offset=tap(sqv).offset,
                                       ap=[[tap(sqv).ap[0][0], PT], [2 * C, ng],
                                           [32, 2 * h], [1, 32]])
                        nc.vector.tensor_reduce(out=nrm[0:PT, 0:ng, 0:2 * h],
                                                in_=sq_h, axis=AX.X, op=OP.add)
                        nc.vector.tensor_scalar(out=nrm[0:PT, 0:ng, 0:2 * h],
                                                in0=nrm[0:PT, 0:ng, 0:2 * h],
                                                scalar1=1e-24, scalar2=None,
                                                op0=OP.max)
                        nc.scalar.activation(nrm[0:PT, 0:ng, 0:2 * h],
                                             nrm[0:PT, 0:ng, 0:2 * h], AF.Sqrt)
                        nc.vector.reciprocal(nrm[0:PT, 0:ng, 0:2 * h],
                                             nrm[0:PT, 0:ng, 0:2 * h])
                        nc.vector.tensor_tensor(
                            nrm[0:PT, 0:ng, 0:h], nrm[0:PT, 0:ng, 0:h],
                            scl[0:PT, None, 0:h].to_broadcast((PT, ng, h)), OP.mult)
                        qn = SP.tile([PT, CG, C], BF16, tag="qnb")
                        kn = SP.tile([PT, CG, C], BF16, tag="knb")
                        for which, dstt in ((0, qn), (1, kn)):
                            srcv = bass.AP(tensor=qkv.tensor,
                                           offset=tap(qkv).offset + which * C,
                                           ap=[[tap(qkv).ap[0][0], PT], [3 * C, ng],
                                               [32, h], [1, 32]])
                            dstv = bass.AP(tensor=dstt.tensor,
                                           offset=tap(dstt).offset,
                                           ap=[[tap(dstt).ap[0][0], PT],
                                               [tap(dstt).ap[1][0], ng],
                                               [32, h], [1, 32]])
                            nrm_v = bass.AP(tensor=nrm.tensor,
                                            offset=tap(nrm).offset + which * h,
                                            ap=[[tap(nrm).ap[0][0], PT], [48, ng],
                                                [1, h], [0, 32]])
                            nc.vector.tensor_tensor(dstv, srcv, nrm_v, OP.mult)
                        HP = cdiv(C, 64)
                        qnT = SP.tile([64, HP, CG, PT], BF16, tag="qnt")
                        knT = SP.tile([64, HP, CG, PT], BF16, tag="knt")
                        for lci in range(ng):
                            for kch in range(Cc):
                                k0, k1 = kch * 128, min((kch + 1) * 128, C)
                                for srcb, dstb in ((qn, qnT), (kn, knT)):
                                    pt = ps.tile([128, 256], BF16, tag="ps")
                                    nc.tensor.transpose(pt[0:k1 - k0, 0:PT],
                                                        srcb[0:PT, lci, k0:k1],
                                                        identb[0:PT, 0:PT])
                                    nc.scalar.copy(dstb[0:64, 2 * kch, lci, :],
                                                   pt[0:64, 0:PT])
                                    if k1 - k0 > 64:
                                        nc.scalar.copy(
                                            dstb[0:k1 - k0 - 64, 2 * kch + 1, lci, :],
                                            pt[64:k1 - k0, 0:PT])
                        # QK + exp + softmax factors
                        Eall = SP2.tile([PT, h, CG, PT], BF16, tag="eall", bufs=1)
                        for hh in range(h):
                            hg, ho = hh // 2, (hh % 2) * 32
                            for lci in range(ng):
                                pq = ps.tile([128, 512], F32, tag="ps")
                                nc.tensor.matmul(pq[0:PT, 0:PT],
                                                 qnT[ho:ho + 32, hg, lci, :],
                                                 knT[ho:ho + 32, hg, lci, :])
                                nc.scalar.activation(Eall[0:PT, hh, lci, :],
                                                     pq[0:PT, 0:PT], AF.Exp,
                                                     bias=neg26[0:PT, :])
                            Eh = Eall[0:PT, hh, 0:ng, :]
                            if s < 3:
                                if shift:
                                    nc.vector.tensor_tensor(
                                        Eh, Eh, m01t[s][0:PT, g0:g0 + ng, :], OP.mult)
                                else:
                                    nc.vector.tensor_tensor(
                                        Eh, Eh,
                                        m01ut[0:PT, None, :].to_broadcast(
                                            (PT, ng, 128)), OP.mult)
                                nc.vector.tensor_tensor(
                                    Eh, Eh,
                                    ebd[hh][0:PT, None, :].to_broadcast((PT, ng, 128)),
                                    OP.mult)
                            else:
                                nc.vector.tensor_tensor(
                                    Eh, Eh,
                                    bass.AP(tensor=eba.tensor,
                                            offset=tap(eba).offset + hh * 64,
                                            ap=[[tap(eba).ap[0][0], PT], [0, 1],
                                                [1, 64]]),
                                    OP.mult)
                            rs = sm.tile([128, 16], F32, tag="rs")
                            Eh_seg = bass.AP(
                                tensor=Eall.tensor,
                                offset=tap(Eall).offset + hh * CG * PT,
                                ap=[[tap(Eall).ap[0][0], PT], [64, ng * WPC],
                                    [1, 64]])
                            nc.vector.tensor_reduce(out=rs[0:PT, 0:ng * WPC],
                                                    in_=Eh_seg, axis=AX.X, op=OP.add)
                            nc.vector.reciprocal(rs[0:PT, 0:ng * WPC],
                                                 rs[0:PT, 0:ng * WPC])
                            rs_v = bass.AP(tensor=rs.tensor, offset=tap(rs).offset,
                                           ap=[[tap(rs).ap[0][0], PT], [1, ng * WPC],
                                               [0, 64]])
                            nc.vector.tensor_tensor(Eh_seg, Eh_seg, rs_v, OP.mult)
                        # P^T + AV -> channel-major attn out
                        aoT = SP.tile([128, Cc, CG, PT], F32R, tag="aot")
                        for lci in range(ng):
                            for kch in range(Cc):
                                k1 = min(128, C - kch * 128)
                                av0 = ps.tile([64, 512], F32, tag="ps", name="av0")
                                av1 = ps.tile([64, 512], F32, tag="ps", name="av1")
                                hlo = kch * 4
                                hhi = min(h, hlo + 4)
                                for hh in range(hlo, hhi):
                                    pt = ps.tile([128, 256], BF16, tag="ps")
                                    nc.tensor.transpose(pt[0:PT, 0:PT],
                                                        Eall[0:PT, hh, lci, :],
                                                        identb[0:PT, 0:PT])
                                    pts = ptp.tile([128, 128], BF16, tag="pts")
                                    nc.scalar.copy(pts[0:PT, 0:PT], pt[0:PT, 0:PT])
                                    vsl = bass.AP(
                                        tensor=qkv.tensor,
                                        offset=tap(qkv).offset + lci * 3 * C + 2 * C
                                        + hh * 32,
                                        ap=[[tap(qkv).ap[0][0], PT], [1, 32]])
                                    pr, po_ = (hh % 4) // 2, (hh % 2) * 32
                                    dst = (av0 if pr == 0 else av1)
                                    nc.tensor.matmul(dst[po_:po_ + 32, 0:PT],
                                                     vsl, pts[0:PT, 0:PT])
                                nc.scalar.copy(aoT[0:min(64, k1), kch, lci, :],
                                               av0[0:min(64, k1), 0:PT])
                                if k1 > 64:
                                    nc.scalar.copy(aoT[64:k1, kch, lci, :],
                                                   av1[0:k1 - 64, 0:PT])
                        # proj + scatter
                        lts2 = [[(aoT[0:min(128, C - kch * 128), kch, lci, :], PT)
                                 for kch in range(Cc)] for lci in range(ng)]
                        po_t = {}

                        def ev_pj(lci, nt, pap, po_t=po_t):
                            if lci not in po_t:
                                po_t[lci] = pop.tile([128, 768], F32, tag="po", name=f"po{lci}")
                            n0 = nt * 512
                            n1 = min(n0 + 512, C)
                            nc.vector.tensor_tensor(po_t[lci][0:PT, n0:n1], pap,
                                                    pjbR[0:PT, n0:n1], OP.add)
                        K.dense_mm(lts2, D[p + "projT"], C, C, ps, wts, ev_pj)
                        for lci, ci in enumerate(cg):
                            for w in range(WPC):
                                wr = (WPC * ci + w) // nWrow
                                wc = (WPC * ci + w) % nWrow
                                nc.sync.dma_start(
                                    out=bass.AP(
                                        tensor=ao,
                                        offset=((wr * WS + shift) * Wp
                                                + wc * WS + shift) * C,
                                        ap=[[Wp * C, WS], [1, WS * C]]),
                                    in_=po_t[lci][w * 64:(w + 1) * 64, 0:C])
                    if shift:
                        K.halo_unfix(ao, H, C)

                    # ---------- phase B ----------
                    gR = K.brep(D[p + "ln1g"], C, bcp, "bC2")
                    bR = K.brep(D[p + "ln1b"], C, bcp, "bC2")
                    gR2 = K.brep(D[p + "ln2g"], C, bcp, "bC3")
                    bR2 = K.brep(D[p + "ln2b"], C, bcp, "bC3")
                    f1bR = K.brep(D[p + "fc1b"], 4 * C, brp, "b4C")
                    f2bR = K.brep(D[p + "fc2b"], C, brp, "bC")
                    for g0 in range(0, Tc, CG):
                        cg = list(range(g0, min(g0 + CG, Tc)))
                        ng = len(cg)
                        h1 = SP.tile([PT, CG, C], F32, tag="h1b")
                        aob = SP.tile([PT, CG, C], F32, tag="aob")
                        for lci, ci in enumerate(cg):
                            r0 = ci * rpc
                            nc.sync.dma_start(
                                out=h1[0:PT, lci, :],
                                in_=bass.AP(tensor=st, offset=r0 * Wp * C,
                                            ap=[[Wp * C, rpc], [1, H * C]]))
                            nc.sync.dma_start(
                                out=aob[0:PT, lci, :],
                                in_=bass.AP(tensor=ao, offset=r0 * Wp * C,
                                            ap=[[Wp * C, rpc], [1, H * C]]))
                        K.ln_cg(aob[0:PT, 0:ng, :], PT, ng, C, gR, bR, sm, sq, eps_t)
                        nc.vector.tensor_tensor(h1[0:PT, 0:ng, :], h1[0:PT, 0:ng, :],
                                                aob[0:PT, 0:ng, :], OP.add)
                        mlp = SP.tile([PT, CG, C], F32, tag="mlpb")
                        lts = []
                        for lci in range(ng):
                            kts = []
                            for kch in range(Cc):
                                k0, k1 = kch * 128, min((kch + 1) * 128, C)
                                xt = K.transpose_to(h1[0:PT, lci, k0:k1], PT, k1 - k0,
                                                    identf, ps, xtp, F32R, "xt")
                                kts.append((xt[0:k1 - k0, 0:PT], PT))
                            lts.append(kts)
                        gm_t = {}

                        def ev_f1(lci, nt, pap, gm_t=gm_t):
                            if lci not in gm_t:
                                gm_t[lci] = SP2.tile([128, 4 * C], F32, tag="gm", name=f"gm{lci}", bufs=2)
                            n0 = nt * 512
                            n1 = min(n0 + 512, 4 * C)
                            nc.vector.tensor_tensor(gm_t[lci][0:PT, n0:n1], pap,
                                                    f1bR[0:PT, n0:n1], OP.add)
                            nc.scalar.activation(gm_t[lci][0:PT, n0:n1],
                                                 gm_t[lci][0:PT, n0:n1], AF.Gelu)
                        K.dense_mm(lts, D[p + "fc1T"], C, 4 * C, ps, wts, ev_f1)
                        NT2 = cdiv(C, 512)
                        for lci in range(ng):
                            ps2 = [ps.tile([128, 512], F32, tag="ps",
                                           name=f"ps2_{_i}") for _i in range(NT2)]
                            for kch in range(C4c):
                                k0, k1 = kch * 128, min((kch + 1) * 128, 4 * C)
                                gt = K.transpose_to(gm_t[lci][0:PT, k0:k1], PT,
                                                    k1 - k0, identf, ps, xtp,
                                                    F32R, "xt")
                                for nt in range(NT2):
                                    n0, n1 = nt * 512, min((nt + 1) * 512, C)
                                    wt = wts.tile([128, 512], F32R, tag="wt")
                                    nc.sync.dma_start(
                                        out=wt[0:k1 - k0, 0:n1 - n0],
                                        in_=bass.AP(tensor=D[p + "fc2T"],
                                                    offset=k0 * C + n0,
                                                    ap=[[C, k1 - k0], [1, n1 - n0]]))
                                    nc.tensor.matmul(ps2[nt][0:PT, 0:n1 - n0],
                                                     gt[0:k1 - k0, 0:PT],
                                                     wt[0:k1 - k0, 0:n1 - n0],
                                                     start=(kch == 0),
                                                     stop=(kch == C4c - 1))
                            for nt in range(NT2):
                                n0, n1 = nt * 512, min((nt + 1) * 512, C)
                                nc.vector.tensor_tensor(mlp[0:PT, lci, n0:n1],
                                                        ps2[nt][0:PT, 0:n1 - n0],
                                                        f2bR[0:PT, n0:n1], OP.add)
                        K.ln_cg(mlp[0:PT, 0:ng, :], PT, ng, C, gR2, bR2, sm, sq,
                                eps_t)
                        nc.vector.tensor_tensor(h1[0:PT, 0:ng, :], h1[0:PT, 0:ng, :],
                                                mlp[0:PT, 0:ng, :], OP.add)
                        for lci, ci in enumerate(cg):
                            r0 = ci * rpc
                            nc.sync.dma_start(
                                out=bass.AP(tensor=st, offset=r0 * Wp * C,
                                            ap=[[Wp * C, rpc], [1, H * C]]),
                                in_=h1[0:PT, lci, :])
                    K.halo_fix(st, H, C)

                # ============ merge ============
                if s < 3:
                    C2, H2 = 2 * C, H // 2
                    T2 = H2 * H2
                    PT2 = min(128, T2)
                    Tc2 = max(1, T2 // 128)
                    rpc2 = PT2 // H2
                    Wp2 = H2 + PAD
                    st2 = D[f"st{s + 1}"]
                    NTm = cdiv(C2, 512)
                    gRm = K.brep(D[f"mrg{s}_g"], C2, bcp, "bC2")
                    bRm = K.brep(D[f"mrg{s}_b"], C2, bcp, "bC2")
                    pw = smI.tile([128, 8], F32, tag="poolm")
                    for g0 in range(0, Tc2, 8):
                        cg = list(range(g0, min(g0 + 8, Tc2)))
                        ng = len(cg)
                        mrg = SP.tile([PT2, min(8, Tc2), C2], F32, tag="mrgb")
                        for lci, ci in enumerate(cg):
                            cat = SP2.tile([128, 4 * C], F32, tag="gm", bufs=2)
                            r0 = ci * rpc2
                            for sl, (dr, dc) in enumerate(((0, 0), (1, 0),
                                                           (0, 1), (1, 1))):
                                nc.sync.dma_start(
                                    out=cat[0:PT2, sl * C:(sl + 1) * C],
                                    in_=bass.AP(
                                        tensor=st,
                                        offset=((2 * r0 + dr) * Wp + dc) * C,
                                        ap=[[2 * Wp * C, rpc2], [2 * C, H2],
                                            [1, C]]))
                            psm = [ps.tile([128, 512], F32, tag="ps",
                                           name=f"psm{_i}") for _i in range(NTm)]
                            nkc = cdiv(4 * C, 128)
                            for kch in range(nkc):
                                k0, k1 = kch * 128, min((kch + 1) * 128, 4 * C)
                                xt = K.transpose_to(cat[0:PT2, k0:k1], PT2, k1 - k0,
                                                    identf, ps, xtp, F32R, "xt")
                                for nt in range(NTm):
                                    n0, n1 = nt * 512, min((nt + 1) * 512, C2)
                                    wt = wts.tile([128, 512], F32R, tag="wt")
                                    nc.sync.dma_start(
                                        out=wt[0:k1 - k0, 0:n1 - n0],
                                        in_=bass.AP(tensor=D[f"mrg{s}_redT"],
                                                    offset=k0 * C2 + n0,
                                                    ap=[[C2, k1 - k0],
                                                        [1, n1 - n0]]))
                                    nc.tensor.matmul(psm[nt][0:PT2, 0:n1 - n0],
                                                     xt[0:k1 - k0, 0:PT2],
                                                     wt[0:k1 - k0, 0:n1 - n0],
                                                     start=(kch == 0),
                                                     stop=(kch == nkc - 1))
                            for nt in range(NTm):
                                n0, n1 = nt * 512, min((nt + 1) * 512, C2)
                                nc.scalar.copy(mrg[0:PT2, lci, n0:n1],
                                               psm[nt][0:PT2, 0:n1 - n0])
                        K.ln_cg(mrg[0:PT2, 0:ng, :], PT2, ng, C2, gRm, bRm, sm, sq,
                                eps_t)
                        nc.vector.tensor_reduce(out=pw[0:PT2, g0:g0 + ng],
                                                in_=mrg[0:PT2, 0:ng, :], axis=AX.X,
                                                op=OP.add)
                        for lci, ci in enumerate(cg):
                            r0 = ci * rpc2
                            nc.sync.dma_start(
                                out=bass.AP(tensor=st2, offset=r0 * Wp2 * C2,
                                            ap=[[Wp2 * C2, rpc2], [1, H2 * C2]]),
                                in_=mrg[0:PT2, lci, :])
                    nc.vector.tensor_scalar_mul(out=pw[0:PT2, 0:Tc2],
                                                in0=pw[0:PT2, 0:Tc2],
                                                scalar1=1.0 / C2)
                    nc.sync.dma_start(out=D[f"o_p{s + 1}"][:, :],
                                      in_=pw[0:PT2, 0:Tc2])
                    K.halo_fix(st2, H2, C2)

        # ============ final head ============
        with tc.tile_pool(name="fin", bufs=1) as FP:
            C = 768
            Wp = 12
            seq = FP.tile([64, 1, C], F32, tag="seq")
            nc.sync.dma_start(
                out=seq[:, 0, :],
                in_=bass.AP(tensor=D["st3"], offset=0, ap=[[Wp * C, 8], [1, 8 * C]]))
            gR = K.brep(D["fing"], C, bcp, "bC2")
            bR = K.brep(D["finb"], C, bcp, "bC2")
            K.ln_cg(seq[:, :, :], 64, 1, C, gR, bR, sm, sq, eps_t)
            pooledT = FP.tile([128, 6], F32, tag="plT")
            pp = ps.tile([128, 512], F32, tag="ps")
            for m in range(6):
                nc.tensor.matmul(pp[0:128, m:m + 1],
                                 seq[:, 0, m * 128:(m + 1) * 128], onesc[0:64, :])
            c64 = K.const_tile(cst, 1.0 / 64.0, "c64")
            nc.scalar.activation(pooledT[:, :], pp[0:128, 0:6], AF.Copy,
                                 scale=c64[:, :])
            hbr = sm.tile([128, 4], F32, tag="hbr")
            nc.sync.dma_start(out=hbr, in_=D["headbr"][:, :])
            ph = ps.tile([128, 512], F32, tag="ps")
            for m in range(4):
                for k in range(6):
                    wt = wts.tile([128, 512], F32, tag="wth", bufs=1)
                    nc.sync.dma_start(
                        out=wt[:, 0:128],
                        in_=bass.AP(tensor=D["headT"],
                                    offset=k * 128 * 512 + m * 128,
                                    ap=[[512, 128], [1, 128]]))
                    nc.tensor.matmul(ph[0:128, m:m + 1], wt[:, 0:128],
                                     pooledT[:, k:k + 1],
                                     start=(k == 0), stop=(k == 5))
            ho = FP.tile([128, 4], F32, tag="ho")
            nc.vector.tensor_tensor(ho[:, :], ph[0:128, 0:4], hbr[:, :], OP.add)
            nc.sync.dma_start(out=D["o_head"][:, :], in_=ho[:, :])

    nc.finalize()
    return nc


_NC = None


def _get_nc():
    global _NC
    if _NC is None:
        _NC = build_nc()
    return _NC


def kernel(x, params):
    import ml_dtypes
    x = np.asarray(x, dtype=np.float32)
    B = x.shape[0]
    f = flatten_params(params)
    consts = host_constants()
    for s in range(3):
        consts[f"m01_{s}"] = consts[f"m01_{s}"].astype(ml_dtypes.bfloat16)
    consts["m01u"] = consts["m01u"].astype(ml_dtypes.bfloat16)
    consts["gmat"] = consts["gmat"].astype(ml_dtypes.bfloat16)

    in_maps = []
    for i in range(B):
        xp = x[i].reshape(3, 64, 4, 64, 4).transpose(1, 3, 0, 2, 4).reshape(4096, 48)
        m = {"xp": np.ascontiguousarray(xp.T)}
        m.update(consts)
        m.update(f)
        in_maps.append(m)

    nc = _get_nc()
    res = bass_utils.run_bass_kernel_spmd(nc, in_maps, list(range(NCORES)))
    heads, p0, p1, p2, p3 = [], [], [], [], []
    for r in res.results:
        heads.append(r["o_head"].T.reshape(512))
        p0.append(r["o_p0"].T.reshape(4096).reshape(1, 64, 64))
        p1.append(r["o_p1"].T.reshape(1024).reshape(1, 32, 32))
        p2.append(r["o_p2"].T.reshape(256).reshape(1, 16, 16))
        p3.append(r["o_p3"].T.reshape(64).reshape(1, 8, 8))
    return (np.stack(heads).astype(np.float32),
            np.stack(p0), np.stack(p1), np.stack(p2), np.stack(p3))
